# revision 23
# baseline (speedup 1.0000x reference)
"""Trainium2 Bass kernel: attention prefill (RoPE + KV-cache + causal SDPA + out proj).

Problem shape (hardcoded): x (2,1024,4096) f32, 32 q-heads / 8 kv-heads, hd=128.

Distribution over 8 NeuronCores:
  - Tensor-parallel over heads for QKV projections + attention: core c owns
    q-heads [4c, 4c+4) and kv-head c.  x is replicated (bf16, pre-transposed).
  - After attention, an AllToAll per half-batch (4 total, issued as each
    half's tiles finish so they overlap compute) reshards attn output from
    head-split to token-split (each core ends with the full 4096-dim attn
    activation for 128 tokens per batch).  Then each core runs the full
    contraction with wo for its tokens, so no all-reduce is needed.
  - KV-cache scatter: device emits RoPE'd k and v per kv-head shard; host
    pastes them into copies of the input caches at `positions`.

Compute in bf16 (PSUM accumulates f32); softmax in f32 on the scalar engine.
"""

import math

import numpy as np
import ml_dtypes

import concourse.bass as bass
import concourse.bacc as bacc
import concourse.tile as tile
from concourse import mybir
from concourse.bass_utils import run_bass_kernel_spmd
from concourse.masks import make_identity

BF16 = ml_dtypes.bfloat16
FP32 = mybir.dt.float32
BF16_DT = mybir.dt.bfloat16

BSZ, SEQ, DIM = 2, 1024, 4096
N_HEADS, N_KV, HD = 32, 8, 128
NC_CORES = 8
HL = N_HEADS // NC_CORES          # 4 local q heads
T = BSZ * SEQ                     # 2048 flat tokens
NT = T // 128                     # 16 token tiles
NTB = SEQ // 128                  # 8 token tiles per batch
DT = DIM // 128                   # 32 contraction tiles
EQ = HL * HD                      # 512 local q-proj cols
SCALE = 1.0 / math.sqrt(HD)
MAX_SEQ = 4096

X_CHUNK = 256                     # tokens per x DMA chunk
NXC = T // X_CHUNK                # 4 chunks


def build_kernel(phase=4):
    nc = bacc.Bacc("TRN2", target_bir_lowering=False, debug=False,
                   num_devices=NC_CORES)

    # ---- I/O -----------------------------------------------------------
    xT = nc.declare_dram_parameter("xT", [T // X_CHUNK, 128, DIM // 128, X_CHUNK], BF16_DT, isOutput=False)
    wq_t = nc.declare_dram_parameter("wq_t", [128, DIM // 128, EQ], BF16_DT, isOutput=False)
    wkv_t = nc.declare_dram_parameter("wkv_t", [128, DIM // 128, 2 * HD], BF16_DT, isOutput=False)
    # wo strips: strip di = wo.T[:, di*256:(di+1)*256], SBUF-image layout
    wo_s = nc.declare_dram_parameter("wo_s", [16, 128, DIM // 128, 256], BF16_DT, isOutput=False)
    cos_q = nc.declare_dram_parameter("cos_q", [SEQ, HL * HD // 2], FP32, isOutput=False)
    sin_q = nc.declare_dram_parameter("sin_q", [SEQ, HL * HD // 2], FP32, isOutput=False)
    cos_k = nc.declare_dram_parameter("cos_k", [SEQ, HD // 2], FP32, isOutput=False)
    sin_k = nc.declare_dram_parameter("sin_k", [SEQ, HD // 2], FP32, isOutput=False)
    # causal tile mask, already divided by SCALE (added to pre-scale scores)
    trimask = nc.declare_dram_parameter("trimask", [128, 128], FP32, isOutput=False)

    out = nc.declare_dram_parameter("out", [BSZ, 128, DIM], FP32, isOutput=True)
    k_out = nc.declare_dram_parameter("k_out", [T, HD], FP32, isOutput=True)
    v_out = nc.declare_dram_parameter("v_out", [T, HD], FP32, isOutput=True)

    xT_r = xT.ap()
    wq_r = wq_t.ap()
    wkv_r = wkv_t.ap()
    wo_r2 = wo_s.ap()
    cos_q_r = cos_q.ap().rearrange("(st p) j -> st p j", p=128)
    sin_q_r = sin_q.ap().rearrange("(st p) j -> st p j", p=128)
    cos_k_r = cos_k.ap().rearrange("(st p) j -> st p j", p=128)
    sin_k_r = sin_k.ap().rearrange("(st p) j -> st p j", p=128)
    k_out_r = k_out.ap().rearrange("(i p) d -> i p d", p=128)
    v_out_r = v_out.ap().rearrange("(i p) d -> i p d", p=128)

    with tile.TileContext(nc) as tc:
        with (
            tc.tile_pool(name="const", bufs=1) as const_pool,
            tc.tile_pool(name="dram", bufs=1, space="DRAM") as dram_pool,
            tc.tile_pool(name="resident", bufs=1) as res_pool,
            tc.tile_pool(name="a2a_sb", bufs=1) as a2a_pool,
            tc.tile_pool(name="wo_sb", bufs=3) as wo_pool,
            tc.tile_pool(name="o_sb", bufs=3) as o_pool,
        ):
            # A2A bounce buffers: one pair per half batch (4 tiles each)
            a2a_in = [[dram_pool.tile([NC_CORES * EQ, 64], BF16_DT,
                                      name=f"a2a_in{b}_{hh}")
                       for hh in range(2)] for b in range(BSZ)]
            a2a_out = [[dram_pool.tile([NC_CORES * EQ, 64], BF16_DT,
                                       name=f"a2a_out{b}_{hh}")
                        for hh in range(2)] for b in range(BSZ)]

            # ---- constants / weights resident in SBUF ------------------
            wq_sb = const_pool.tile([128, DT, EQ], BF16_DT)
            wkv_sb = const_pool.tile([128, DT, 2 * HD], BF16_DT)
            cq_sb = const_pool.tile([128, NTB, HL * HD // 2], FP32)
            nc.scalar.dma_start(out=cq_sb, in_=cos_q_r.rearrange("st p j -> p st j"))
            sq_sb = const_pool.tile([128, NTB, HL * HD // 2], FP32)
            nc.scalar.dma_start(out=sq_sb, in_=sin_q_r.rearrange("st p j -> p st j"))
            ck_sb = const_pool.tile([128, NTB, HD // 2], FP32)
            nc.scalar.dma_start(out=ck_sb, in_=cos_k_r.rearrange("st p j -> p st j"))
            sk_sb = const_pool.tile([128, NTB, HD // 2], FP32)
            nc.scalar.dma_start(out=sk_sb, in_=sin_k_r.rearrange("st p j -> p st j"))
            tri_sb = const_pool.tile([128, 128], FP32)
            nc.scalar.dma_start(out=tri_sb, in_=trimask.ap())
            ident_sb = const_pool.tile([128, 128], BF16_DT)
            make_identity(nc, ident_sb)

            # RoPE'd K^T (hd x tokens) and V (tokens x hd), resident bf16
            kT_sb = res_pool.tile([128, T], BF16_DT)
            v_sb = res_pool.tile([128, NT, HD], BF16_DT)

            a2a_sb = []
            with (
                tc.tile_pool(name="xchunk", bufs=2) as x_pool,
                tc.tile_pool(name="work", bufs=3) as work,
                tc.tile_pool(name="probs", bufs=2) as probs_pool,
                tc.tile_pool(name="pt", bufs=4) as pt_pool,
                tc.tile_pool(name="attn", bufs=2) as attn_pool,
                tc.tile_pool(name="ps_q", bufs=1, space="PSUM") as ps_q,
                tc.tile_pool(name="ps_kv", bufs=1, space="PSUM") as ps_kv,
                tc.tile_pool(name="ps_s", bufs=2, space="PSUM") as ps_s,
                tc.tile_pool(name="ps_tp", bufs=2, space="PSUM") as ps_tp,
            ):
                # ramp-up: x chunk 0 / wq / wkv interleaved in need-order
                x_sb = x_pool.tile([128, DT, X_CHUNK], BF16_DT,
                                   name="x_sb", tag="x_sb")
                nc.sync.dma_start(out=x_sb[:, 0:8, :], in_=xT_r[0, :, 0:8, :])
                nc.sync.dma_start(out=wq_sb[:, 0:8, :], in_=wq_r[:, 0:8, :])
                for c4 in range(1, 4):
                    nc.sync.dma_start(out=x_sb[:, c4 * 8:(c4 + 1) * 8, :],
                                      in_=xT_r[0, :, c4 * 8:(c4 + 1) * 8, :])
                    nc.sync.dma_start(out=wq_sb[:, c4 * 8:(c4 + 1) * 8, :],
                                      in_=wq_r[:, c4 * 8:(c4 + 1) * 8, :])
                    nc.sync.dma_start(
                        out=wkv_sb[:, (c4 - 1) * 11:min(DT, c4 * 11), :],
                        in_=wkv_r[:, (c4 - 1) * 11:min(DT, c4 * 11), :])
                for b in range(BSZ):
                    for qi in range(NTB):
                        i = b * NTB + qi          # global token tile
                        if i % (X_CHUNK // 128) == 0 and i > 0:
                            x_sb = x_pool.tile([128, DT, X_CHUNK], BF16_DT,
                                               name="x_sb", tag="x_sb")
                            nc.sync.dma_start(
                                out=x_sb, in_=xT_r[(i * 128) // X_CHUNK])
                        xcol = (i * 128) % X_CHUNK

                        # ---- QKV projections --------------------------
                        q_ps = ps_q.tile([128, EQ], FP32, name="q_ps")
                        kv_ps = ps_kv.tile([128, 2 * HD], FP32, name="kv_ps")
                        for dt_i in range(DT):
                            nc.tensor.matmul(q_ps, x_sb[:, dt_i, xcol:xcol + 128],
                                             wq_sb[:, dt_i, :],
                                             start=(dt_i == 0), stop=(dt_i == DT - 1))
                        for dt_i in range(DT):
                            nc.tensor.matmul(kv_ps, x_sb[:, dt_i, xcol:xcol + 128],
                                             wkv_sb[:, dt_i, :],
                                             start=(dt_i == 0), stop=(dt_i == DT - 1))

                        # ---- RoPE on q (4 heads at once) --------------
                        # pairs: (re, im) interleaved along free dim
                        q_re = q_ps.rearrange("p (h j two) -> p (h j) two",
                                              h=HL, two=2)[:, :, 0]
                        q_im = q_ps.rearrange("p (h j two) -> p (h j) two",
                                              h=HL, two=2)[:, :, 1]
                        cq = cq_sb[:, qi, :]
                        sq = sq_sb[:, qi, :]
                        t1 = work.tile([128, EQ // 2], FP32, name="t1")
                        t2 = work.tile([128, EQ // 2], FP32, name="t2")
                        q_roped = work.tile([128, EQ], BF16_DT, name="q_roped")
                        qr_re = q_roped.rearrange("p (h j two) -> p (h j) two",
                                                  h=HL, two=2)[:, :, 0]
                        qr_im = q_roped.rearrange("p (h j two) -> p (h j) two",
                                                  h=HL, two=2)[:, :, 1]
                        nc.vector.tensor_mul(t1, q_re, cq)
                        nc.vector.tensor_mul(t2, q_im, sq)
                        nc.vector.tensor_sub(qr_re, t1, t2)
                        nc.vector.tensor_mul(t1, q_re, sq)
                        nc.vector.tensor_mul(t2, q_im, cq)
                        nc.vector.tensor_add(qr_im, t1, t2)

                        # ---- RoPE on k (f32 for cache, bf16 for attn) -
                        k_re = kv_ps[:, 0:HD].rearrange("p (j two) -> p j two",
                                                        two=2)[:, :, 0]
                        k_im = kv_ps[:, 0:HD].rearrange("p (j two) -> p j two",
                                                        two=2)[:, :, 1]
                        ckt = ck_sb[:, qi, :]
                        skt = sk_sb[:, qi, :]
                        t3 = work.tile([128, HD // 2], FP32, name="t3")
                        t4 = work.tile([128, HD // 2], FP32, name="t4")
                        k_f32 = work.tile([128, HD], FP32, name="k_f32")
                        kf_re = k_f32.rearrange("p (j two) -> p j two",
                                                two=2)[:, :, 0]
                        kf_im = k_f32.rearrange("p (j two) -> p j two",
                                                two=2)[:, :, 1]
                        nc.vector.tensor_mul(t3, k_re, ckt)
                        nc.vector.tensor_mul(t4, k_im, skt)
                        nc.vector.tensor_sub(kf_re, t3, t4)
                        nc.vector.tensor_mul(t3, k_re, skt)
                        nc.vector.tensor_mul(t4, k_im, ckt)
                        nc.vector.tensor_add(kf_im, t3, t4)
                        k_bf = work.tile([128, HD], BF16_DT, name="k_bf")
                        nc.vector.tensor_copy(k_bf, k_f32)
                        nc.scalar.dma_start(out=k_out_r[i], in_=k_f32)

                        # ---- V: copy out (f32 cache + bf16 resident) --
                        v_f32 = work.tile([128, HD], FP32, name="v_f32")
                        nc.vector.tensor_copy(v_f32, kv_ps[:, HD:2 * HD])
                        nc.vector.tensor_copy(v_sb[:, i, :], kv_ps[:, HD:2 * HD])
                        nc.scalar.dma_start(out=v_out_r[i], in_=v_f32)

                        # ---- transposes: qT per head, kT (on PE) -------
                        qT = work.tile([128, HL, 128], BF16_DT, name="qT")
                        for h in range(HL):
                            tq_ps = ps_tp.tile([128, 128], BF16_DT, name="tp_ps",
                                               tag="tp_ps")
                            nc.tensor.transpose(
                                tq_ps, q_roped[:, h * HD:(h + 1) * HD], ident_sb)
                            nc.vector.tensor_copy(qT[:, h, :], tq_ps)
                        tk_ps = ps_tp.tile([128, 128], BF16_DT, name="tp_ps",
                                           tag="tp_ps")
                        nc.tensor.transpose(tk_ps, k_bf, ident_sb)
                        nc.vector.tensor_copy(
                            kT_sb[:, i * 128:(i + 1) * 128], tk_ps)

                        # ---- attention for this q tile -----------------
                        if phase < 2:
                            continue
                        L = (qi + 1) * 128
                        kbase = b * SEQ
                        def mk_scores(h):
                            s_ps = ps_s.tile([128, NTB * 128], FP32,
                                             name="s_ps", tag="s_ps")
                            for s0 in range(0, L, 512):
                                sl = min(512, L - s0)
                                nc.tensor.matmul(
                                    s_ps[:, s0:s0 + sl], qT[:, h, :],
                                    kT_sb[:, kbase + s0:kbase + s0 + sl],
                                    start=True, stop=True)
                            # causal mask on the diagonal tile (pre-scale)
                            nc.vector.tensor_add(
                                s_ps[:, qi * 128:L], s_ps[:, qi * 128:L], tri_sb)
                            return s_ps

                        s_next = mk_scores(0)
                        for h in range(HL):
                            s_ps = s_next
                            if h + 1 < HL:
                                s_next = mk_scores(h + 1)

                            probs = probs_pool.tile([128, NTB * 128], BF16_DT,
                                                    name="probs", tag="probs")
                            rsum = work.tile([128, 1], FP32, name="rsum")
                            nc.scalar.activation(
                                out=probs[:, 0:L], in_=s_ps[:, 0:L],
                                func=mybir.ActivationFunctionType.Exp,
                                scale=SCALE, accum_out=rsum)
                            rinv = work.tile([128, 1], FP32, name="rinv")
                            nc.vector.reciprocal(rinv, rsum)
                            nc.vector.tensor_scalar_mul(
                                probs[:, 0:L], probs[:, 0:L], rinv)

                            if phase < 3:
                                continue
                            # PV: accumulate over k tiles into psum slice
                            pv_ps = s_ps[:, 0:128]
                            for j in range(qi + 1):
                                tp_ps = ps_tp.tile([128, 128], BF16_DT,
                                                   name="tp_ps", tag="tp_ps")
                                nc.tensor.transpose(
                                    tp_ps, probs[:, j * 128:(j + 1) * 128],
                                    ident_sb)
                                pT = pt_pool.tile([128, 128], BF16_DT, name="pT",
                                                  tag="pTb")
                                nc.vector.tensor_copy(pT, tp_ps)
                                nc.tensor.matmul(
                                    pv_ps, v_sb[:, b * NTB + j, :], pT,
                                    start=(j == 0), stop=(j == qi),
                                    skip_group_check=True)
                            attnT = attn_pool.tile([128, 128], BF16_DT,
                                                   name="attnT")
                            nc.vector.tensor_copy(attnT, pv_ps)
                            hh, lt = qi // 4, qi % 4
                            for qq in range(2):
                                nc.scalar.dma_start(
                                    out=a2a_in[b][hh][
                                        (2 * lt + qq) * EQ + h * HD:
                                        (2 * lt + qq) * EQ + (h + 1) * HD, :],
                                    in_=attnT[:, qq * 64:(qq + 1) * 64])

                        if phase >= 3 and qi % 4 == 3:
                            hh = qi // 4
                            nc.gpsimd.collective_compute(
                                "AllToAll", mybir.AluOpType.bypass,
                                replica_groups=[list(range(NC_CORES))],
                                ins=[a2a_in[b][hh].opt()],
                                outs=[a2a_out[b][hh].opt()])
                            if phase >= 4:
                                if hh == 0:
                                    t_ = a2a_pool.tile(
                                        [128, DT, 128], BF16_DT,
                                        name=f"a2a_sb{b}")
                                    a2a_sb.append(t_)
                                nc.gpsimd.dma_start(
                                    out=a2a_sb[b][:, :, hh * 64:(hh + 1) * 64],
                                    in_=a2a_out[b][hh].rearrange(
                                        "(et p) t -> p et t", p=128))



            # ---- output projection (token-sharded, full contraction) ---
            if phase >= 4:
                with (
                    tc.tile_pool(name="ps_o", bufs=3, space="PSUM") as ps_o,
                ):
                    def do_wo(di, b, wst):
                        o_ps = ps_o.tile([128, 256], FP32, name="o_ps",
                                         tag="o_ps")
                        for et in range(DT):
                            nc.tensor.matmul(o_ps, a2a_sb[b][:, et, :],
                                             wst[:, et, :],
                                             start=(et == 0), stop=(et == DT - 1))
                        o_sb = o_pool.tile([128, 256], FP32, name="o_sb",
                                           tag="o_sb")
                        nc.vector.tensor_copy(o_sb, o_ps)
                        nc.scalar.dma_start(
                            out=out.ap()[b, :, di * 256:(di + 1) * 256],
                            in_=o_sb)

                    for di in range(16):
                        wst = wo_pool.tile([128, DT, 256], BF16_DT, name="wst",
                                           tag="wst")
                        nc.sync.dma_start(out=wst, in_=wo_r2[di])
                        do_wo(di, 0, wst)
                        do_wo(di, 1, wst)
    nc.finalize()
    return nc


_CACHE = {}


def _get_kernel():
    if "nc" not in _CACHE:
        _CACHE["nc"] = build_kernel()
    return _CACHE["nc"]


def kernel(x, positions, mask, wq, wk, wv, wo, freqs_cos, freqs_sin,
           cache_k, cache_v, _trace=False):
    nc = _get_kernel()

    x = np.asarray(x, np.float32)
    mask = np.asarray(mask, np.float32)
    wq = np.asarray(wq, np.float32)
    wk = np.asarray(wk, np.float32)
    wv = np.asarray(wv, np.float32)
    wo = np.asarray(wo, np.float32)
    freqs_cos = np.asarray(freqs_cos, np.float32)
    freqs_sin = np.asarray(freqs_sin, np.float32)
    positions = np.asarray(positions)

    # device kernel assumes causal masking: 0 on/below diagonal, <=-1e8 above
    tri = np.tril(np.ones((SEQ, SEQ), bool))
    assert (mask[tri] == 0).all() and (mask[~tri] <= -1e8).all(), \
        "kernel compiled for causal mask"

    x2d = x.reshape(T, DIM)
    # SBUF-image layouts: [.., 128 partitions, contiguous free bytes]
    x_swz = np.ascontiguousarray(
        x2d.astype(BF16).reshape(T // X_CHUNK, X_CHUNK, DT, 128)
        .transpose(0, 3, 2, 1))
    wqT = wq.astype(BF16).T            # (DIM, N_HEADS*HD)
    wkT = wk.astype(BF16).T
    wvT = wv.astype(BF16).T
    woT = wo.astype(BF16).T            # (e, d)
    wo_strips = np.ascontiguousarray(
        woT.reshape(DT, 128, 16, 256).transpose(2, 1, 0, 3))
    cos_q = np.ascontiguousarray(np.tile(freqs_cos, (1, HL)))
    sin_q = np.ascontiguousarray(np.tile(freqs_sin, (1, HL)))
    trimask = np.ascontiguousarray(mask[:128, :128] / SCALE)

    in_maps = []
    for c in range(NC_CORES):
        in_maps.append({
            "xT": x_swz,
            "wq_t": np.ascontiguousarray(
                wqT[:, c * EQ:(c + 1) * EQ].reshape(DT, 128, EQ)
                .transpose(1, 0, 2)),
            "wkv_t": np.ascontiguousarray(np.concatenate(
                [wkT[:, c * HD:(c + 1) * HD], wvT[:, c * HD:(c + 1) * HD]],
                axis=1).reshape(DT, 128, 2 * HD).transpose(1, 0, 2)),
            "wo_s": wo_strips,
            "cos_q": cos_q,
            "sin_q": sin_q,
            "cos_k": np.ascontiguousarray(freqs_cos),
            "sin_k": np.ascontiguousarray(freqs_sin),
            "trimask": trimask,
        })

    res = run_bass_kernel_spmd(nc, in_maps, core_ids=list(range(NC_CORES)),
                               trace=_trace)
    results = res.results

    out2d = np.empty((T, DIM), np.float32)
    xk = np.empty((BSZ, SEQ, N_KV, HD), np.float32)
    xv = np.empty((BSZ, SEQ, N_KV, HD), np.float32)
    for c in range(NC_CORES):
        r = results[c]
        o = np.asarray(r["out"], np.float32)        # (BSZ, 128, DIM)
        for b in range(BSZ):
            for hh in range(2):
                t0 = b * SEQ + hh * (SEQ // 2) + 64 * c
                out2d[t0:t0 + 64] = o[b, hh * 64:(hh + 1) * 64]
        xk[:, :, c, :] = np.asarray(r["k_out"], np.float32).reshape(BSZ, SEQ, HD)
        xv[:, :, c, :] = np.asarray(r["v_out"], np.float32).reshape(BSZ, SEQ, HD)

    out = out2d.reshape(BSZ, SEQ, DIM)
    pos = np.asarray(positions) % MAX_SEQ
    cache_k_new = np.array(cache_k, np.float32, copy=True)
    cache_v_new = np.array(cache_v, np.float32, copy=True)
    cache_k_new[:, pos] = xk
    cache_v_new[:, pos] = xv

    if _trace:
        return (out, cache_k_new, cache_v_new), res
    return out, cache_k_new, cache_v_new



# revision 24
# speedup vs baseline: 1.1312x; 1.1312x over previous
"""Trainium2 Bass kernel: attention prefill (RoPE + KV-cache + causal SDPA + out proj).

Problem shape (hardcoded): x (2,1024,4096) f32, 32 q-heads / 8 kv-heads, hd=128.

Distribution over 8 NeuronCores:
  - Tensor-parallel over heads for QKV projections + attention: core c owns
    q-heads [4c, 4c+4) and kv-head c.  x is replicated (bf16, pre-transposed).
  - After attention, an AllToAll per half-batch (4 total, issued as each
    half's tiles finish so they overlap compute) reshards attn output from
    head-split to token-split (each core ends with the full 4096-dim attn
    activation for 128 tokens per batch).  Then each core runs the full
    contraction with wo for its tokens, so no all-reduce is needed.
  - KV-cache scatter: device emits RoPE'd k and v per kv-head shard; host
    pastes them into copies of the input caches at `positions`.

Compute in bf16 (PSUM accumulates f32); softmax in f32 on the scalar engine.
"""

import math

import numpy as np
import ml_dtypes

import concourse.bass as bass
import concourse.bacc as bacc
import concourse.tile as tile
from concourse import mybir
from concourse.bass_utils import run_bass_kernel_spmd
from concourse.masks import make_identity

BF16 = ml_dtypes.bfloat16
FP32 = mybir.dt.float32
BF16_DT = mybir.dt.bfloat16

BSZ, SEQ, DIM = 2, 1024, 4096
N_HEADS, N_KV, HD = 32, 8, 128
NC_CORES = 8
HL = N_HEADS // NC_CORES          # 4 local q heads
T = BSZ * SEQ                     # 2048 flat tokens
NT = T // 128                     # 16 token tiles
NTB = SEQ // 128                  # 8 token tiles per batch
DT = DIM // 128                   # 32 contraction tiles
EQ = HL * HD                      # 512 local q-proj cols
SCALE = 1.0 / math.sqrt(HD)
MAX_SEQ = 4096

X_CHUNK = 256                     # tokens per x DMA chunk
NXC = T // X_CHUNK                # 4 chunks


def build_kernel(phase=4):
    nc = bacc.Bacc("TRN2", target_bir_lowering=False, debug=False,
                   num_devices=NC_CORES)

    # ---- I/O -----------------------------------------------------------
    xT = nc.declare_dram_parameter("xT", [T // X_CHUNK, 128, DIM // 128, X_CHUNK], BF16_DT, isOutput=False)
    wq_t = nc.declare_dram_parameter("wq_t", [128, DIM // 128, EQ], BF16_DT, isOutput=False)
    wkv_t = nc.declare_dram_parameter("wkv_t", [128, DIM // 128, 2 * HD], BF16_DT, isOutput=False)
    # wo strips: strip di = wo.T[:, di*256:(di+1)*256], SBUF-image layout
    wo_s = nc.declare_dram_parameter("wo_s", [8, 128, DIM // 128, 512], BF16_DT, isOutput=False)
    cos_q = nc.declare_dram_parameter("cos_q", [SEQ, HL * HD // 2], BF16_DT, isOutput=False)
    sin_q = nc.declare_dram_parameter("sin_q", [SEQ, HL * HD // 2], BF16_DT, isOutput=False)
    cos_k = nc.declare_dram_parameter("cos_k", [SEQ, HD // 2], BF16_DT, isOutput=False)
    sin_k = nc.declare_dram_parameter("sin_k", [SEQ, HD // 2], BF16_DT, isOutput=False)
    # causal tile mask, already divided by SCALE (added to pre-scale scores)
    trimask = nc.declare_dram_parameter("trimask", [128, 128], FP32, isOutput=False)

    out = nc.declare_dram_parameter("out", [BSZ, 128, DIM], FP32, isOutput=True)
    k_out = nc.declare_dram_parameter("k_out", [T, HD], FP32, isOutput=True)
    v_out = nc.declare_dram_parameter("v_out", [T, HD], FP32, isOutput=True)

    xT_r = xT.ap()
    wq_r = wq_t.ap()
    wkv_r = wkv_t.ap()
    wo_r2 = wo_s.ap()
    cos_q_r = cos_q.ap().rearrange("(st p) j -> st p j", p=128)
    sin_q_r = sin_q.ap().rearrange("(st p) j -> st p j", p=128)
    cos_k_r = cos_k.ap().rearrange("(st p) j -> st p j", p=128)
    sin_k_r = sin_k.ap().rearrange("(st p) j -> st p j", p=128)
    k_out_r = k_out.ap().rearrange("(i p) d -> i p d", p=128)
    v_out_r = v_out.ap().rearrange("(i p) d -> i p d", p=128)

    with tile.TileContext(nc) as tc:
        with (
            tc.tile_pool(name="const", bufs=1) as const_pool,
            tc.tile_pool(name="dram", bufs=1, space="DRAM") as dram_pool,
            tc.tile_pool(name="resident", bufs=1) as res_pool,
            tc.tile_pool(name="a2a_sb", bufs=1) as a2a_pool,
            tc.tile_pool(name="wo_sb", bufs=2) as wo_pool,
            tc.tile_pool(name="o_sb", bufs=3) as o_pool,
        ):
            # A2A bounce buffers: one pair per half batch (4 tiles each)
            a2a_in = [[dram_pool.tile([NC_CORES * EQ, 64], BF16_DT,
                                      name=f"a2a_in{b}_{hh}")
                       for hh in range(2)] for b in range(BSZ)]
            a2a_out = [[dram_pool.tile([NC_CORES * EQ, 64], BF16_DT,
                                       name=f"a2a_out{b}_{hh}")
                        for hh in range(2)] for b in range(BSZ)]

            # ---- constants / weights resident in SBUF ------------------
            wq_sb = const_pool.tile([128, DT, EQ], BF16_DT)
            wkv_sb = const_pool.tile([128, DT, 2 * HD], BF16_DT)
            cq_sb = const_pool.tile([128, NTB, HL * HD // 2], BF16_DT)
            nc.scalar.dma_start(out=cq_sb, in_=cos_q_r.rearrange("st p j -> p st j"))
            sq_sb = const_pool.tile([128, NTB, HL * HD // 2], BF16_DT)
            nc.scalar.dma_start(out=sq_sb, in_=sin_q_r.rearrange("st p j -> p st j"))
            ck_sb = const_pool.tile([128, NTB, HD // 2], BF16_DT)
            nc.scalar.dma_start(out=ck_sb, in_=cos_k_r.rearrange("st p j -> p st j"))
            sk_sb = const_pool.tile([128, NTB, HD // 2], BF16_DT)
            nc.scalar.dma_start(out=sk_sb, in_=sin_k_r.rearrange("st p j -> p st j"))
            tri_sb = const_pool.tile([128, 128], FP32)
            nc.scalar.dma_start(out=tri_sb, in_=trimask.ap())
            ident_sb = const_pool.tile([128, 128], BF16_DT)
            make_identity(nc, ident_sb)

            # RoPE'd K^T (hd x tokens) and V (tokens x hd), resident bf16
            kT_sb = res_pool.tile([128, T], BF16_DT)
            v_sb = res_pool.tile([128, NT, HD], BF16_DT)

            a2a_sb = []
            with (
                tc.tile_pool(name="xchunk", bufs=2) as x_pool,
                tc.tile_pool(name="work", bufs=3) as work,
                tc.tile_pool(name="probs", bufs=2) as probs_pool,
                tc.tile_pool(name="pt", bufs=4) as pt_pool,
                tc.tile_pool(name="attn", bufs=2) as attn_pool,
                tc.tile_pool(name="ps_q", bufs=1, space="PSUM") as ps_q,
                tc.tile_pool(name="ps_kv", bufs=1, space="PSUM") as ps_kv,
                tc.tile_pool(name="ps_s", bufs=2, space="PSUM") as ps_s,
                tc.tile_pool(name="ps_tp", bufs=2, space="PSUM") as ps_tp,
            ):
                # ramp-up: x chunk 0 / wq / wkv interleaved in need-order
                x_sb = x_pool.tile([128, DT, X_CHUNK], BF16_DT,
                                   name="x_sb", tag="x_sb")
                nc.sync.dma_start(out=x_sb[:, 0:8, :], in_=xT_r[0, :, 0:8, :])
                nc.sync.dma_start(out=wq_sb[:, 0:8, :], in_=wq_r[:, 0:8, :])
                for c4 in range(1, 4):
                    nc.sync.dma_start(out=x_sb[:, c4 * 8:(c4 + 1) * 8, :],
                                      in_=xT_r[0, :, c4 * 8:(c4 + 1) * 8, :])
                    nc.sync.dma_start(out=wq_sb[:, c4 * 8:(c4 + 1) * 8, :],
                                      in_=wq_r[:, c4 * 8:(c4 + 1) * 8, :])
                    nc.sync.dma_start(
                        out=wkv_sb[:, (c4 - 1) * 11:min(DT, c4 * 11), :],
                        in_=wkv_r[:, (c4 - 1) * 11:min(DT, c4 * 11), :])
                for b in range(BSZ):
                    for qi in range(NTB):
                        i = b * NTB + qi          # global token tile
                        if i % (X_CHUNK // 128) == 0 and i > 0:
                            x_sb = x_pool.tile([128, DT, X_CHUNK], BF16_DT,
                                               name="x_sb", tag="x_sb")
                            nc.sync.dma_start(
                                out=x_sb, in_=xT_r[(i * 128) // X_CHUNK])
                        xcol = (i * 128) % X_CHUNK

                        # ---- QKV projections --------------------------
                        q_ps = ps_q.tile([128, EQ], FP32, name="q_ps")
                        kv_ps = ps_kv.tile([128, 2 * HD], FP32, name="kv_ps")
                        for dt_i in range(DT):
                            nc.tensor.matmul(q_ps, x_sb[:, dt_i, xcol:xcol + 128],
                                             wq_sb[:, dt_i, :],
                                             start=(dt_i == 0), stop=(dt_i == DT - 1))
                        for dt_i in range(DT):
                            nc.tensor.matmul(kv_ps, x_sb[:, dt_i, xcol:xcol + 128],
                                             wkv_sb[:, dt_i, :],
                                             start=(dt_i == 0), stop=(dt_i == DT - 1))

                        # ---- RoPE on q (4 heads at once) --------------
                        # pairs: (re, im) interleaved along free dim
                        q_re = q_ps.rearrange("p (h j two) -> p (h j) two",
                                              h=HL, two=2)[:, :, 0]
                        q_im = q_ps.rearrange("p (h j two) -> p (h j) two",
                                              h=HL, two=2)[:, :, 1]
                        cq = cq_sb[:, qi, :]
                        sq = sq_sb[:, qi, :]
                        t1 = work.tile([128, EQ // 2], FP32, name="t1")
                        t2 = work.tile([128, EQ // 2], FP32, name="t2")
                        q_roped = work.tile([128, EQ], BF16_DT, name="q_roped")
                        qr_re = q_roped.rearrange("p (h j two) -> p (h j) two",
                                                  h=HL, two=2)[:, :, 0]
                        qr_im = q_roped.rearrange("p (h j two) -> p (h j) two",
                                                  h=HL, two=2)[:, :, 1]
                        nc.vector.tensor_mul(t1, q_re, cq)
                        nc.vector.tensor_mul(t2, q_im, sq)
                        nc.vector.tensor_sub(qr_re, t1, t2)
                        nc.vector.tensor_mul(t1, q_re, sq)
                        nc.vector.tensor_mul(t2, q_im, cq)
                        nc.vector.tensor_add(qr_im, t1, t2)

                        # ---- RoPE on k (f32 for cache, bf16 for attn) -
                        k_re = kv_ps[:, 0:HD].rearrange("p (j two) -> p j two",
                                                        two=2)[:, :, 0]
                        k_im = kv_ps[:, 0:HD].rearrange("p (j two) -> p j two",
                                                        two=2)[:, :, 1]
                        ckt = ck_sb[:, qi, :]
                        skt = sk_sb[:, qi, :]
                        t3 = work.tile([128, HD // 2], FP32, name="t3")
                        t4 = work.tile([128, HD // 2], FP32, name="t4")
                        k_f32 = work.tile([128, HD], FP32, name="k_f32")
                        kf_re = k_f32.rearrange("p (j two) -> p j two",
                                                two=2)[:, :, 0]
                        kf_im = k_f32.rearrange("p (j two) -> p j two",
                                                two=2)[:, :, 1]
                        nc.vector.tensor_mul(t3, k_re, ckt)
                        nc.vector.tensor_mul(t4, k_im, skt)
                        nc.vector.tensor_sub(kf_re, t3, t4)
                        nc.vector.tensor_mul(t3, k_re, skt)
                        nc.vector.tensor_mul(t4, k_im, ckt)
                        nc.vector.tensor_add(kf_im, t3, t4)
                        k_bf = work.tile([128, HD], BF16_DT, name="k_bf")
                        nc.vector.tensor_copy(k_bf, k_f32)
                        nc.scalar.dma_start(out=k_out_r[i], in_=k_f32)

                        # ---- V: copy out (f32 cache + bf16 resident) --
                        v_f32 = work.tile([128, HD], FP32, name="v_f32")
                        nc.vector.tensor_copy(v_f32, kv_ps[:, HD:2 * HD])
                        nc.vector.tensor_copy(v_sb[:, i, :], kv_ps[:, HD:2 * HD])
                        nc.scalar.dma_start(out=v_out_r[i], in_=v_f32)

                        # ---- transposes: qT per head, kT (on PE) -------
                        qT = work.tile([128, HL, 128], BF16_DT, name="qT")
                        for h in range(HL):
                            tq_ps = ps_tp.tile([128, 128], BF16_DT, name="tp_ps",
                                               tag="tp_ps")
                            nc.tensor.transpose(
                                tq_ps, q_roped[:, h * HD:(h + 1) * HD], ident_sb)
                            nc.vector.tensor_copy(qT[:, h, :], tq_ps)
                        tk_ps = ps_tp.tile([128, 128], BF16_DT, name="tp_ps",
                                           tag="tp_ps")
                        nc.tensor.transpose(tk_ps, k_bf, ident_sb)
                        nc.vector.tensor_copy(
                            kT_sb[:, i * 128:(i + 1) * 128], tk_ps)

                        # ---- attention for this q tile -----------------
                        if phase < 2:
                            continue
                        L = (qi + 1) * 128
                        kbase = b * SEQ
                        def mk_scores(h):
                            s_ps = ps_s.tile([128, NTB * 128], FP32,
                                             name="s_ps", tag="s_ps")
                            for s0 in range(0, L, 512):
                                sl = min(512, L - s0)
                                nc.tensor.matmul(
                                    s_ps[:, s0:s0 + sl], qT[:, h, :],
                                    kT_sb[:, kbase + s0:kbase + s0 + sl],
                                    start=True, stop=True)
                            # causal mask on the diagonal tile (pre-scale)
                            nc.vector.tensor_add(
                                s_ps[:, qi * 128:L], s_ps[:, qi * 128:L], tri_sb)
                            return s_ps

                        s_next = mk_scores(0)
                        for h in range(HL):
                            s_ps = s_next
                            if h + 1 < HL:
                                s_next = mk_scores(h + 1)

                            probs = probs_pool.tile([128, NTB * 128], BF16_DT,
                                                    name="probs", tag="probs")
                            rsum = work.tile([128, 1], FP32, name="rsum")
                            nc.scalar.activation(
                                out=probs[:, 0:L], in_=s_ps[:, 0:L],
                                func=mybir.ActivationFunctionType.Exp,
                                scale=SCALE, accum_out=rsum)
                            rinv = work.tile([128, 1], FP32, name="rinv")
                            nc.vector.reciprocal(rinv, rsum)
                            nc.vector.tensor_scalar_mul(
                                probs[:, 0:L], probs[:, 0:L], rinv)

                            if phase < 3:
                                continue
                            # PV: accumulate over k tiles into psum slice
                            pv_ps = s_ps[:, 0:128]
                            for j in range(qi + 1):
                                tp_ps = ps_tp.tile([128, 128], BF16_DT,
                                                   name="tp_ps", tag="tp_ps")
                                nc.tensor.transpose(
                                    tp_ps, probs[:, j * 128:(j + 1) * 128],
                                    ident_sb)
                                pT = pt_pool.tile([128, 128], BF16_DT, name="pT",
                                                  tag="pTb")
                                nc.vector.tensor_copy(pT, tp_ps)
                                nc.tensor.matmul(
                                    pv_ps, v_sb[:, b * NTB + j, :], pT,
                                    start=(j == 0), stop=(j == qi),
                                    skip_group_check=True)
                            attnT = attn_pool.tile([128, 128], BF16_DT,
                                                   name="attnT")
                            nc.vector.tensor_copy(attnT, pv_ps)
                            hh, lt = qi // 4, qi % 4
                            for qq in range(2):
                                nc.scalar.dma_start(
                                    out=a2a_in[b][hh][
                                        (2 * lt + qq) * EQ + h * HD:
                                        (2 * lt + qq) * EQ + (h + 1) * HD, :],
                                    in_=attnT[:, qq * 64:(qq + 1) * 64])

                        if phase >= 3 and qi % 4 == 3:
                            hh = qi // 4
                            nc.gpsimd.collective_compute(
                                "AllToAll", mybir.AluOpType.bypass,
                                replica_groups=[list(range(NC_CORES))],
                                ins=[a2a_in[b][hh].opt()],
                                outs=[a2a_out[b][hh].opt()])
                            if phase >= 4:
                                if hh == 0:
                                    t_ = a2a_pool.tile(
                                        [128, DT, 128], BF16_DT,
                                        name=f"a2a_sb{b}")
                                    a2a_sb.append(t_)
                                nc.gpsimd.dma_start(
                                    out=a2a_sb[b][:, :, hh * 64:(hh + 1) * 64],
                                    in_=a2a_out[b][hh].rearrange(
                                        "(et p) t -> p et t", p=128))



            # ---- output projection (token-sharded, full contraction) ---
            if phase >= 4:
                with (
                    tc.tile_pool(name="ps_o", bufs=3, space="PSUM") as ps_o,
                ):
                    def do_wo(di, b, wst):
                        o_ps = ps_o.tile([128, 512], FP32, name="o_ps",
                                         tag="o_ps")
                        for et in range(DT):
                            nc.tensor.matmul(o_ps, a2a_sb[b][:, et, :],
                                             wst[:, et, :],
                                             start=(et == 0), stop=(et == DT - 1))
                        o_sb = o_pool.tile([128, 512], FP32, name="o_sb",
                                           tag="o_sb")
                        nc.vector.tensor_copy(o_sb, o_ps)
                        nc.scalar.dma_start(
                            out=out.ap()[b, :, di * 512:(di + 1) * 512],
                            in_=o_sb)

                    for di in range(8):
                        wst = wo_pool.tile([128, DT, 512], BF16_DT, name="wst",
                                           tag="wst")
                        nc.sync.dma_start(out=wst[:, 0:DT // 2, :],
                                          in_=wo_r2[di, :, 0:DT // 2, :])
                        nc.sync.dma_start(out=wst[:, DT // 2:, :],
                                          in_=wo_r2[di, :, DT // 2:, :])
                        do_wo(di, 0, wst)
                        do_wo(di, 1, wst)
    nc.finalize()
    return nc


_CACHE = {}


def _get_kernel():
    if "nc" not in _CACHE:
        _CACHE["nc"] = build_kernel()
    return _CACHE["nc"]


def kernel(x, positions, mask, wq, wk, wv, wo, freqs_cos, freqs_sin,
           cache_k, cache_v, _trace=False):
    nc = _get_kernel()

    x = np.asarray(x, np.float32)
    mask = np.asarray(mask, np.float32)
    wq = np.asarray(wq, np.float32)
    wk = np.asarray(wk, np.float32)
    wv = np.asarray(wv, np.float32)
    wo = np.asarray(wo, np.float32)
    freqs_cos = np.asarray(freqs_cos, np.float32)
    freqs_sin = np.asarray(freqs_sin, np.float32)
    positions = np.asarray(positions)

    # device kernel assumes causal masking: 0 on/below diagonal, <=-1e8 above
    tri = np.tril(np.ones((SEQ, SEQ), bool))
    assert (mask[tri] == 0).all() and (mask[~tri] <= -1e8).all(), \
        "kernel compiled for causal mask"

    x2d = x.reshape(T, DIM)
    # SBUF-image layouts: [.., 128 partitions, contiguous free bytes]
    x_swz = np.ascontiguousarray(
        x2d.astype(BF16).reshape(T // X_CHUNK, X_CHUNK, DT, 128)
        .transpose(0, 3, 2, 1))
    wqT = wq.astype(BF16).T            # (DIM, N_HEADS*HD)
    wkT = wk.astype(BF16).T
    wvT = wv.astype(BF16).T
    woT = wo.astype(BF16).T            # (e, d)
    wo_strips = np.ascontiguousarray(
        woT.reshape(DT, 128, 8, 512).transpose(2, 1, 0, 3))
    cos_q = np.ascontiguousarray(np.tile(freqs_cos, (1, HL)).astype(BF16))
    sin_q = np.ascontiguousarray(np.tile(freqs_sin, (1, HL)).astype(BF16))
    trimask = np.ascontiguousarray(mask[:128, :128] / SCALE)

    in_maps = []
    for c in range(NC_CORES):
        in_maps.append({
            "xT": x_swz,
            "wq_t": np.ascontiguousarray(
                wqT[:, c * EQ:(c + 1) * EQ].reshape(DT, 128, EQ)
                .transpose(1, 0, 2)),
            "wkv_t": np.ascontiguousarray(np.concatenate(
                [wkT[:, c * HD:(c + 1) * HD], wvT[:, c * HD:(c + 1) * HD]],
                axis=1).reshape(DT, 128, 2 * HD).transpose(1, 0, 2)),
            "wo_s": wo_strips,
            "cos_q": cos_q,
            "sin_q": sin_q,
            "cos_k": np.ascontiguousarray(freqs_cos.astype(BF16)),
            "sin_k": np.ascontiguousarray(freqs_sin.astype(BF16)),
            "trimask": trimask,
        })

    res = run_bass_kernel_spmd(nc, in_maps, core_ids=list(range(NC_CORES)),
                               trace=_trace)
    results = res.results

    out2d = np.empty((T, DIM), np.float32)
    xk = np.empty((BSZ, SEQ, N_KV, HD), np.float32)
    xv = np.empty((BSZ, SEQ, N_KV, HD), np.float32)
    for c in range(NC_CORES):
        r = results[c]
        o = np.asarray(r["out"], np.float32)        # (BSZ, 128, DIM)
        for b in range(BSZ):
            for hh in range(2):
                t0 = b * SEQ + hh * (SEQ // 2) + 64 * c
                out2d[t0:t0 + 64] = o[b, hh * 64:(hh + 1) * 64]
        xk[:, :, c, :] = np.asarray(r["k_out"], np.float32).reshape(BSZ, SEQ, HD)
        xv[:, :, c, :] = np.asarray(r["v_out"], np.float32).reshape(BSZ, SEQ, HD)

    out = out2d.reshape(BSZ, SEQ, DIM)
    pos = np.asarray(positions) % MAX_SEQ
    cache_k_new = np.array(cache_k, np.float32, copy=True)
    cache_v_new = np.array(cache_v, np.float32, copy=True)
    cache_k_new[:, pos] = xk
    cache_v_new[:, pos] = xv

    if _trace:
        return (out, cache_k_new, cache_v_new), res
    return out, cache_k_new, cache_v_new

